# revision 22
# baseline (speedup 1.0000x reference)
"""Trainium2 Bass kernel for nn_Architecture_51161650430159 (3-node ConvGRU graph net).

Key algebraic structure (exact, not approximate):
  - The recurrence starts from zero state, so in sweep 0 the two big
    td_proj matmuls see zero input: td0 = td_b0, td1 = td_b1.
  - Sweep-0 nodes 1 and 2 get x=0, h=0, so their outputs are the
    per-channel constants sigmoid(gates_b)*tanh(can_b).
  - When can_b[1] == can_b[2] == 0 and td_b0 == td_b1 == 0 (which the
    problem's input spec guarantees: all biases are zeros), those states
    are exactly 0 and the 12544x6272 td weights NEVER affect the output.
  The computation then collapses to 4 ConvGRU cells + the FC head, which
  this kernel evaluates on-device (bf16 matmuls, fp32 PSUM + fp32
  post-sigmoid math), batch-sharded over the 8 NeuronCores (2 samples
  per core, no collectives needed).

Host-side work is limited to sharding / layout permutation of inputs and
concatenation of outputs; all arithmetic runs on the NeuronCores.

Conv layout: input maps live in "arenas" -- bf16 tiles (72, BL, 30, 30)
whose partition rows are 9 blocks of 8 channels, one per 3x3 tap (dy, dx).
Block 4 (dy=1, dx=1) is the "mid" block at partitions [32:40] (quadrant-
aligned, so engines may write it); the other 8 blocks are filled by
SBUF-SBUF shift DMAs (DMA has no partition-alignment restriction).  A 3x3
conv is then one K=72 matmul per 392-pixel chunk, accumulated over input-
map arenas in PSUM.  One 16-wide sigmoid ACT per conv per image reads PSUM
[u|cand] (tanh(x) = 2*sigmoid(2x)-1 with the 2x folded into weights and
biases); the cand half is extracted to a partition-0 tile by DMA because
SBUF engine operands must start at partition 0/32/64/96.
"""

import os
import numpy as np

LAST_EXEC_NS = None
LAST_TRACE_DIR = None
LAST_RESULTS = None

_CACHE = {}

B, HD, H, W = 16, 8, 28, 28
NCORES = 8
BL = B // NCORES

# tap order: mid first, then the 8 shifted taps
TAPS = [(1, 1), (0, 0), (0, 1), (0, 2), (1, 0), (1, 2), (2, 0), (2, 1), (2, 2)]
# partition row offset of each tap block inside an arena (K = 72, gap-free).
# Taps at rows 32/64 are quadrant-aligned so DVE copies them; the other six
# are filled by SBUF-SBUF DMA (exempt from partition alignment).
BLOCK_OFF = [0, 8, 16, 24, 32, 40, 48, 56, 64]
KA = 72

# wpk column offsets: (name, off, M)
WREG = dict(xia=(0, 8), a0u=(8, 8), a0c=(16, 8), g01x=(24, 16),
            g01h=(40, 16), c01x=(56, 8), c01r=(64, 8), a1u=(72, 8),
            a1c=(80, 8), m2u=(88, 8), s11c=(96, 8))
WPK_COLS = 104


def build_fast_nc():
    import concourse.bacc as bacc
    import concourse.tile as tile
    import concourse.mybir as mybir
    from concourse.masks import make_identity

    f32 = mybir.dt.float32
    bf16 = mybir.dt.bfloat16
    AF = mybir.ActivationFunctionType
    OP = mybir.AluOpType

    nc = bacc.Bacc("TRN2", target_bir_lowering=False, debug=False,
                   num_devices=NCORES)

    xin_e = nc.declare_dram_parameter("xin", [3, BL, H, W], f32, isOutput=False)
    td_e = nc.declare_dram_parameter("td8", [HD, BL, H, W], f32, isOutput=False)
    wpk_e = nc.declare_dram_parameter("wpk", [KA, WPK_COLS], f32, isOutput=False)
    bias_e = nc.declare_dram_parameter("biasp", [16, 18], f32, isOutput=False)
    w100_e = nc.declare_dram_parameter("w100", [100, 11], f32, isOutput=False)
    w1_e = nc.declare_dram_parameter("w1h", [128, 8, 7, 100], f32, isOutput=False)
    out_e = nc.declare_dram_parameter("out", [BL, 10], f32, isOutput=True)

    with tile.TileContext(nc) as tc, \
            tc.tile_pool(name="sb", bufs=1) as _sb:
        def _tile(shape, dtype, name):
            return _sb.tile(shape, dtype, tag=name, name=name)

        # ---- arenas: one tile per image so the two image-chains pipeline ----
        XIA = _tile([KA, BL, 30, 30], bf16, name="XIA")

        def img_arena(nm):
            return [_tile([KA, 1, 30, 30], bf16, name=f"{nm}{b}")
                    for b in range(BL)]

        XA = img_arena("XA")
        HA = img_arena("HA")
        RA = img_arena("RA")
        S01A = img_arena("S01A")
        SGTA = img_arena("SGTA")
        M2A = img_arena("M2A")
        S11A = img_arena("S11A")

        # ---- staging / weights ----
        xif = _tile([3, BL, H, W], f32, name="xif")
        tdf = _tile([HD, BL, H, W], f32, name="tdf")
        wraw = _tile([KA, WPK_COLS], f32, name="wraw")
        biasT = _tile([16, 18], f32, name="biasT")
        w100f = _tile([100, 11], f32, name="w100f")
        w1f = _tile([128, 8, 7, 100], f32, name="w1f")
        w1b = _tile([128, 8, 7, 100], bf16, name="w1b")
        w2tb = _tile([100, 10], bf16, name="w2tb")

        wts = {}
        for nm, (off, M) in WREG.items():
            wts[nm] = _tile([KA, M], bf16, name=f"wt_{nm}")

        # ---- activations / temps (bf16, per-image free slices) ----
        Ua = _tile([8, 1568], bf16, name="Ua")
        Ca = _tile([8, 1568], bf16, name="Ca")
        S16b = _tile([16, 1568], bf16, name="S16b")
        Ub8 = _tile([8, 1568], bf16, name="Ub8")
        Sb = _tile([8, 1568], bf16, name="Sb")
        t1 = _tile([8, 1568], bf16, name="t1")
        t2 = _tile([8, 1568], bf16, name="t2")
        Uc = _tile([8, 1568], bf16, name="Uc")
        Cc = _tile([8, 1568], bf16, name="Cc")
        Ud = _tile([8, 1568], bf16, name="Ud")
        Cd = _tile([8, 1568], bf16, name="Cd")
        S2a = _tile([8, 1568], bf16, name="S2a")

        TT = _tile([128, 7, 8, BL], bf16, name="TT")
        ident = _tile([8, 8], bf16, name="ident")
        relu1 = _tile([100, BL], bf16, name="relu1")
        outs = _tile([BL, 10], f32, name="outs")

        # ---- input DMAs (sync ring, dependency order; w1 comes later) ----
        nc.sync.dma_start(out=xif[:], in_=xin_e[:])
        nc.sync.dma_start(out=wraw[:], in_=wpk_e[:])
        nc.sync.dma_start(out=biasT[:], in_=bias_e[:])
        nc.sync.dma_start(out=tdf[:], in_=td_e[:])
        nc.sync.dma_start(out=w100f[:], in_=w100_e[:])

        # ---- preload ACT LUT tables (sigmoid + tanh) before they gate ----
        dummy = _tile([1, 4], f32, name="dummy")
        nc.gpsimd.memset(dummy[:], 0.0)
        nc.scalar.activation(dummy[:], dummy[:], AF.Sigmoid)
        nc.scalar.activation(dummy[:], dummy[:], AF.Tanh)

        # ---- early work ----
        nc.vector.memset(XIA[:], 0.0)
        nc.vector.tensor_copy(XIA[0:3, :, 1:29, 1:29], xif[:])

        def wv(nm):
            off, M = WREG[nm]
            return wraw[0:KA, off:off + M]

        def wcast(nm, scale):
            nc.vector.tensor_scalar(wts[nm][:], wv(nm), float(scale), None,
                                    OP.mult)

        wcast("xia", 1.0)
        for b in range(BL):
            nc.gpsimd.memset(SGTA[b][:], 0.0)
            nc.gpsimd.memset(S11A[b][:], 0.0)
        nc.gpsimd.memset(TT[:], 0.0)
        make_identity(nc, ident[:])

        def pad_memsets(arr):
            nc.gpsimd.memset(arr[0:8, :, 0, :], 0.0)
            nc.gpsimd.memset(arr[0:8, :, 29, :], 0.0)
            nc.gpsimd.memset(arr[0:8, :, 1:29, 0], 0.0)
            nc.gpsimd.memset(arr[0:8, :, 1:29, 29], 0.0)

        for pair in (XA, HA, RA, S01A):
            for b in range(BL):
                pad_memsets(pair[b])

        wcast("a0u", 0.5)
        wcast("a0c", 1.0)
        wcast("g01x", 0.5)
        wcast("g01h", 0.5)
        wcast("c01x", 1.0)
        wcast("c01r", 1.0)
        wcast("a1u", 0.4)
        wcast("a1c", 0.8)
        wcast("m2u", 0.7)
        wcast("s11c", 0.7)
        nc.vector.tensor_copy(w2tb[:], w100f[0:100, 0:10])

        # ---- helpers ----
        DMA_BLOCKS = [1, 2, 3, 5, 6, 7]
        DVE_BLOCKS = [4, 8]
        SHIFT_ENGINES = [nc.sync, nc.gpsimd, nc.scalar, nc.sync, nc.gpsimd,
                         nc.scalar]

        def mid_int(arr):
            return arr[0:8, :, 1:29, 1:29]

        def shifts(arr):
            """Fill tap blocks of a one-image arena from its mid block."""
            flat = arr.rearrange("p b r w -> p (b r w)")
            n = 900
            for ei, k in enumerate(DMA_BLOCKS):
                dy, dx = TAPS[k]
                s = 30 * (dy - 1) + (dx - 1)
                L = n - abs(s)
                d0 = max(0, -s)
                s0 = max(0, s)
                p = BLOCK_OFF[k]
                SHIFT_ENGINES[ei].dma_start(out=flat[p:p + 8, d0:d0 + L],
                                            in_=flat[0:8, s0:s0 + L])
            for k in DVE_BLOCKS:
                dy, dx = TAPS[k]
                s = 30 * (dy - 1) + (dx - 1)
                L = n - abs(s)
                d0 = max(0, -s)
                s0 = max(0, s)
                p = BLOCK_OFF[k]
                nc.vector.tensor_copy(flat[p:p + 8, d0:d0 + L],
                                      flat[0:8, s0:s0 + L])

        def conv1(ps, groups, full_arena=None):
            """Per-image conv: groups = [(arena_pair, wt, row0, M)].
            full_arena: use one (KA, BL, ...) arena for both images."""
            for ci in range(4):
                bi, h0 = ci // 2, (ci % 2) * 14
                seen = {}
                for (pair, wnm, row0, M) in groups:
                    seen.setdefault(row0, []).append((pair, wnm, M))
                for row0, glist in seen.items():
                    for gi, (pair, wnm, M) in enumerate(glist):
                        if pair is None:
                            rhs = full_arena[:, bi, 1 + h0:15 + h0, 1:29]
                        else:
                            rhs = pair[bi][:, 0, 1 + h0:15 + h0, 1:29]
                        nc.tensor.matmul(
                            ps[row0:row0 + M, ci, 0:392],
                            wts[wnm][:, :],
                            rhs,
                            start=(gi == 0), stop=(gi == len(glist) - 1),
                        )

        def psb(ps, p0, p1, b):
            return ps[p0:p1, 2 * b:2 * b + 2, 0:392]

        def half(t, b, P=8):
            return t[0:P, b * 784:(b + 1) * 784]

        with tc.tile_pool(name="cps", bufs=2, space="PSUM") as cps:
            # ---- input conv ----
            shifts_full = shifts  # alias for clarity

            flatXIA = XIA.rearrange("p b r w -> p (b r w)")
            n2 = BL * 900
            for ei, k in enumerate(DMA_BLOCKS):
                dy, dx = TAPS[k]
                s = 30 * (dy - 1) + (dx - 1)
                L = n2 - abs(s)
                d0 = max(0, -s)
                s0 = max(0, s)
                p = BLOCK_OFF[k]
                SHIFT_ENGINES[ei].dma_start(out=flatXIA[p:p + 8, d0:d0 + L],
                                            in_=flatXIA[0:8, s0:s0 + L])
            for k in DVE_BLOCKS:
                dy, dx = TAPS[k]
                s = 30 * (dy - 1) + (dx - 1)
                L = n2 - abs(s)
                d0 = max(0, -s)
                s0 = max(0, s)
                p = BLOCK_OFF[k]
                nc.vector.tensor_copy(flatXIA[p:p + 8, d0:d0 + L],
                                      flatXIA[0:8, s0:s0 + L])

            ps0 = cps.tile([40, 4, 512], f32, tag="cp", name="ps0")
            conv1(ps0, [(None, "xia", 0, 8)], full_arena=XIA)
            for b in range(BL):
                nc.scalar.activation(mid_int(XA[b]), psb(ps0, 0, 8, b),
                                     AF.Identity, bias=biasT[0:8, 0:1])
                shifts(XA[b])
                # topdown sigmoid arena for this image (off critical path)
                nc.scalar.activation(SGTA[b][0:8, :, 1:29, 1:29],
                                     tdf[0:HD, b, :, :], AF.Sigmoid)
                shifts(SGTA[b])

            # ---- GRU0 sweep0: s00 = u * cand ----
            ps1 = cps.tile([40, 4, 512], f32, tag="cp", name="ps1")
            conv1(ps1, [(XA, "a0u", 0, 8), (XA, "a0c", 32, 8)])
            for b in range(BL):
                nc.scalar.activation(half(Ua, b), psb(ps1, 0, 8, b),
                                     AF.Sigmoid, bias=biasT[0:8, 1:2])
                nc.scalar.activation(half(Ca, b), psb(ps1, 32, 40, b),
                                     AF.Tanh, bias=biasT[0:8, 3:4])
                nc.vector.tensor_tensor(mid_int(HA[b]), half(Ua, b),
                                        half(Ca, b), OP.mult)
                shifts(HA[b])

            # ---- GRU0 sweep1 gates ----
            ps2 = cps.tile([40, 4, 512], f32, tag="cp", name="ps2")
            conv1(ps2, [(XA, "g01x", 0, 16), (HA, "g01h", 0, 16)])
            for b in range(BL):
                nc.scalar.activation(half(S16b, b, 16), psb(ps2, 0, 16, b),
                                     AF.Sigmoid, bias=biasT[0:16, 2:3])
                nc.sync.dma_start(out=half(Ub8, b),
                                  in_=S16b[8:16, b * 784:(b + 1) * 784])
                nc.vector.tensor_tensor(mid_int(RA[b]), half(S16b, b),
                                        mid_int(HA[b]), OP.mult)
                shifts(RA[b])
            # stream in the first half of fc1 weights while PE is busy
            nc.sync.dma_start(out=w1f[0:64, :, :, :], in_=w1_e[0:64, :, :, :])

            # ---- GRU0 sweep1 cand + update ----
            ps3 = cps.tile([40, 4, 512], f32, tag="cp", name="ps3")
            conv1(ps3, [(XA, "c01x", 0, 8), (RA, "c01r", 0, 8)])
            for b in range(BL):
                nc.scalar.activation(half(Sb, b), psb(ps3, 0, 8, b), AF.Tanh,
                                     bias=biasT[0:8, 3:4])
                nc.vector.tensor_tensor(half(t1, b), half(Sb, b),
                                        mid_int(HA[b]), OP.subtract)
                nc.vector.tensor_tensor(half(t2, b), half(Ub8, b),
                                        half(t1, b), OP.mult)
                nc.vector.tensor_tensor(mid_int(S01A[b]), mid_int(HA[b]),
                                        half(t2, b), OP.add)
                shifts(S01A[b])

            # ---- GRU1 sweep1 ----
            ps4 = cps.tile([40, 4, 512], f32, tag="cp", name="ps4")
            conv1(ps4, [(S01A, "a1u", 0, 8), (S01A, "a1c", 32, 8)])
            for b in range(BL):
                nc.scalar.activation(half(Uc, b), psb(ps4, 0, 8, b),
                                     AF.Sigmoid, bias=biasT[0:8, 4:5])
                nc.scalar.activation(half(Cc, b), psb(ps4, 32, 40, b),
                                     AF.Tanh, bias=biasT[0:8, 5:6])
                nc.vector.tensor_tensor(mid_int(S11A[b]), half(Uc, b),
                                        half(Cc, b), OP.mult)
                shifts(S11A[b])
                # m-arena = s11-arena * sigmoid(td)-arena, all taps at once
                nc.vector.tensor_tensor(M2A[b][0:KA, :, :, :],
                                        S11A[b][0:KA, :, :, :],
                                        SGTA[b][0:KA, :, :, :], OP.mult)
            nc.sync.dma_start(out=w1f[64:128, :, :, :],
                              in_=w1_e[64:128, :, :, :])
            nc.vector.tensor_copy(w1b[:], w1f[:])

            # ---- GRU2 sweep1 (u from m; cand from s11) ----
            ps5 = cps.tile([40, 4, 512], f32, tag="cp", name="ps5")
            conv1(ps5, [(M2A, "m2u", 0, 8), (S11A, "s11c", 32, 8)])
            for b in range(BL):
                nc.scalar.activation(half(Ud, b), psb(ps5, 0, 8, b),
                                     AF.Sigmoid, bias=biasT[0:8, 6:7])
                nc.scalar.activation(half(Cd, b), psb(ps5, 32, 40, b),
                                     AF.Tanh, bias=biasT[0:8, 7:8])
                nc.vector.tensor_tensor(half(S2a, b), half(Ud, b),
                                        half(Cd, b), OP.mult)

        # ---- FC head (relu folded into the transpose copy-out) ----
        with tc.tile_pool(name="tps", bufs=6, space="PSUM") as tps, \
             tc.tile_pool(name="hps", bufs=1, space="PSUM") as hps:
            p1 = hps.tile([100, BL], f32, tag="p1", name="p1")
            idx = 0
            for r in range(7):
                n = 128 if r < 6 else 784 - 6 * 128
                for b in range(BL):
                    tp = tps.tile([128, 8], bf16, tag="tp", name=f"tp{b}{r}")
                    nc.tensor.transpose(
                        tp[0:n, 0:8],
                        S2a[:, b * 784 + 128 * r: b * 784 + 128 * r + n],
                        ident[:])
                    if b == 0:
                        nc.scalar.activation(TT[0:n, r, :, b], tp[0:n, 0:8],
                                             AF.Relu)
                    else:
                        nc.vector.tensor_scalar_max(TT[0:n, r, :, b],
                                                    tp[0:n, 0:8], 0.0)
                for c8 in range(8):
                    nc.tensor.matmul(
                        p1[:, :],
                        w1b[:, c8, r, :],
                        TT[:, r, c8, :],
                        start=(idx == 0), stop=(idx == 55),
                    )
                    idx += 1
            nc.scalar.activation(relu1[:], p1[:], AF.Relu,
                                 bias=w100f[0:100, 10:11])
            p2 = hps.tile([BL, 10], f32, tag="p2", name="p2")
            nc.tensor.matmul(p2[:, :], relu1[:], w2tb[:], start=True, stop=True)
            nc.vector.tensor_tensor(outs[:], p2[:, :], biasT[0:BL, 8:18],
                                    OP.add)

        nc.sync.dma_start(out=out_e[:], in_=outs[:])

    nc.finalize()
    return nc


def prep_shared(inputs):
    f = lambda k: np.ascontiguousarray(np.asarray(inputs[k], np.float32))
    input_conv_w = f("input_conv_w")
    gates_w = f("gates_w")
    can_w = f("can_w")
    gates_b = f("gates_b")
    can_b = f("can_b")
    input_conv_b = f("input_conv_b")
    fc1_w = f("fc1_w")
    fc1_b = f("fc1_b")
    fc2_w = f("fc2_w")
    fc2_b = f("fc2_b")

    def re9(w):
        # (O, C<=8, 3, 3) -> (KA, O): tap k's rows at BLOCK_OFF[k]
        O, C = w.shape[0], w.shape[1]
        a = w.transpose(2, 3, 1, 0)  # (ky, kx, c, o)
        out = np.zeros((KA, O), np.float32)
        for k, (dy, dx) in enumerate(TAPS):
            out[BLOCK_OFF[k]:BLOCK_OFF[k] + C] = a[dy, dx]
        return out

    wpk = np.zeros((KA, WPK_COLS), np.float32)

    def put(nm, arr):
        off, M = WREG[nm]
        assert arr.shape == (KA, M), (nm, arr.shape)
        wpk[:, off:off + M] = arr

    put("xia", re9(input_conv_w))
    put("a0u", re9(gates_w[0][8:16, :8]))
    put("a0c", re9(can_w[0][:, :8]))
    put("g01x", re9(gates_w[0][:, 0:8]))
    put("g01h", re9(gates_w[0][:, 8:16]))
    put("c01x", re9(can_w[0][:, 0:8]))
    put("c01r", re9(can_w[0][:, 8:16]))
    put("a1u", re9(gates_w[1][8:16, :8]))
    put("a1c", re9(can_w[1][:, :8]))
    put("m2u", re9(gates_w[2][8:16, 0:8]))
    put("s11c", re9(can_w[2][:, 0:8]))

    biasp = np.zeros((16, 18), np.float32)
    biasp[0:8, 0] = input_conv_b
    biasp[0:8, 1] = gates_b[0][8:16]
    biasp[0:8, 2] = gates_b[0][0:8]
    biasp[8:16, 2] = gates_b[0][8:16]
    biasp[0:8, 3] = can_b[0]
    biasp[0:8, 4] = gates_b[1][8:16]
    biasp[0:8, 5] = can_b[1]
    biasp[0:8, 6] = gates_b[2][8:16]
    biasp[0:8, 7] = can_b[2]
    biasp[0:BL, 8:18] = fc2_b[None, :]

    w100 = np.zeros((100, 11), np.float32)
    w100[:, 0:10] = fc2_w.T
    w100[:, 10] = fc1_b

    w1r = fc1_w.reshape(100, 8, 784)
    w1h = np.zeros((128, 8, 7, 100), np.float32)
    for r in range(7):
        n = min(128, 784 - 128 * r)
        w1h[:n, :, r, :] = w1r[:, :, 128 * r:128 * r + n].transpose(2, 1, 0)

    return dict(wpk=wpk, biasp=biasp, w100=w100,
                w1h=np.ascontiguousarray(w1h))


def _fast_path_ok(inputs):
    z = lambda k: not np.any(np.asarray(inputs[k]))
    return (z("td_b0") and z("td_b1")
            and not np.any(np.asarray(inputs["can_b"])[1])
            and not np.any(np.asarray(inputs["can_b"])[2]))


def kernel(**inputs):
    global LAST_EXEC_NS, LAST_TRACE_DIR, LAST_RESULTS
    from concourse.bass_utils import run_bass_kernel_spmd

    if not _fast_path_ok(inputs):
        raise NotImplementedError(
            "general-bias path not implemented (spec guarantees zero biases)")

    if "nc" not in _CACHE:
        _CACHE["nc"] = build_fast_nc()
    nc = _CACHE["nc"]

    shared = prep_shared(inputs)
    it = np.asarray(inputs["input_tensor"], np.float32)
    td = np.asarray(inputs["topdown_input"], np.float32)

    in_maps = []
    for c in range(NCORES):
        b0 = c * BL
        xin = np.ascontiguousarray(
            it[b0:b0 + BL, :, 0].transpose(1, 0, 2, 3))        # (3, BL, H, W)
        td8 = np.ascontiguousarray(
            td[b0:b0 + BL, :HD].transpose(1, 0, 2, 3))         # (HD, BL, H, W)
        in_maps.append(dict(xin=xin, td8=td8, **shared))

    trace = bool(int(os.environ.get("KBENCH_TRACE", "0")))
    tmpdir = None
    if trace:
        import tempfile
        tmpdir = tempfile.mkdtemp(prefix="kbench_trace_")
    res = run_bass_kernel_spmd(nc, in_maps, core_ids=list(range(NCORES)),
                               trace=trace, tmpdir=tmpdir)
    LAST_EXEC_NS = res.exec_time_ns
    LAST_TRACE_DIR = tmpdir
    LAST_RESULTS = res
    out = np.concatenate([np.asarray(r["out"], np.float32)
                          for r in res.results], 0)
    return out


# revision 24
# speedup vs baseline: 1.0928x; 1.0928x over previous
"""Trainium2 Bass kernel for nn_Architecture_51161650430159 (3-node ConvGRU graph net).

Key algebraic structure (exact, not approximate):
  - The recurrence starts from zero state, so in sweep 0 the two big
    td_proj matmuls see zero input: td0 = td_b0, td1 = td_b1.
  - Sweep-0 nodes 1 and 2 get x=0, h=0, so their outputs are the
    per-channel constants sigmoid(gates_b)*tanh(can_b).
  - When can_b[1] == can_b[2] == 0 and td_b0 == td_b1 == 0 (which the
    problem's input spec guarantees: all biases are zeros), those states
    are exactly 0 and the 12544x6272 td weights NEVER affect the output.
  The computation then collapses to 4 ConvGRU cells + the FC head, which
  this kernel evaluates on-device (bf16 matmuls, fp32 PSUM + fp32
  post-sigmoid math), batch-sharded over the 8 NeuronCores (2 samples
  per core, no collectives needed).

Host-side work is limited to sharding / layout permutation of inputs and
concatenation of outputs; all arithmetic runs on the NeuronCores.

Conv layout: input maps live in "arenas" -- bf16 tiles (72, BL, 30, 30)
whose partition rows are 9 blocks of 8 channels, one per 3x3 tap (dy, dx).
Block 4 (dy=1, dx=1) is the "mid" block at partitions [32:40] (quadrant-
aligned, so engines may write it); the other 8 blocks are filled by
SBUF-SBUF shift DMAs (DMA has no partition-alignment restriction).  A 3x3
conv is then one K=72 matmul per 392-pixel chunk, accumulated over input-
map arenas in PSUM.  One 16-wide sigmoid ACT per conv per image reads PSUM
[u|cand] (tanh(x) = 2*sigmoid(2x)-1 with the 2x folded into weights and
biases); the cand half is extracted to a partition-0 tile by DMA because
SBUF engine operands must start at partition 0/32/64/96.
"""

import os
import numpy as np

LAST_EXEC_NS = None
LAST_TRACE_DIR = None
LAST_RESULTS = None

_CACHE = {}

B, HD, H, W = 16, 8, 28, 28
NCORES = 8
BL = B // NCORES

# tap order: mid first, then the 8 shifted taps
TAPS = [(1, 1), (0, 0), (0, 1), (0, 2), (1, 0), (1, 2), (2, 0), (2, 1), (2, 2)]
# partition row offset of each tap block inside an arena (K = 72, gap-free).
# Taps at rows 32/64 are quadrant-aligned so DVE copies them; the other six
# are filled by SBUF-SBUF DMA (exempt from partition alignment).
BLOCK_OFF = [0, 8, 16, 24, 32, 40, 48, 56, 64]
KA = 72

# wpk column offsets: (name, off, M)
WREG = dict(xia=(0, 8), a0u=(8, 8), a0c=(16, 8), g01x=(24, 16),
            g01h=(40, 16), c01x=(56, 8), c01r=(64, 8), a1u=(72, 8),
            a1c=(80, 8), m2u=(88, 8), s11c=(96, 8))
WPK_COLS = 104


def build_fast_nc():
    import concourse.bacc as bacc
    import concourse.tile as tile
    import concourse.mybir as mybir
    from concourse.masks import make_identity

    f32 = mybir.dt.float32
    bf16 = mybir.dt.bfloat16
    AF = mybir.ActivationFunctionType
    OP = mybir.AluOpType

    nc = bacc.Bacc("TRN2", target_bir_lowering=False, debug=False,
                   num_devices=NCORES)

    xin_e = nc.declare_dram_parameter("xin", [3, BL, H, W], f32, isOutput=False)
    td_e = nc.declare_dram_parameter("td8", [HD, BL, H, W], f32, isOutput=False)
    wpk_e = nc.declare_dram_parameter("wpk", [KA, WPK_COLS], f32, isOutput=False)
    bias_e = nc.declare_dram_parameter("biasp", [16, 18], f32, isOutput=False)
    w100_e = nc.declare_dram_parameter("w100", [100, 11], f32, isOutput=False)
    w1_e = nc.declare_dram_parameter("w1h", [128, 8, 7, 100], f32, isOutput=False)
    out_e = nc.declare_dram_parameter("out", [BL, 10], f32, isOutput=True)

    with tile.TileContext(nc) as tc, \
            tc.tile_pool(name="sb", bufs=1) as _sb:
        def _tile(shape, dtype, name):
            return _sb.tile(shape, dtype, tag=name, name=name)

        # ---- arenas (partition rows per BLOCK_OFF; [56:64),[72:96) zero) ----
        XIA = _tile([KA, BL, 30, 30], bf16, name="XIA")
        XA = _tile([KA, BL, 30, 30], bf16, name="XA")
        HA = _tile([KA, BL, 30, 30], bf16, name="HA")
        RA = _tile([KA, BL, 30, 30], bf16, name="RA")
        S01A = _tile([KA, BL, 30, 30], bf16, name="S01A")
        SGTA = _tile([KA, BL, 30, 30], bf16, name="SGTA")
        M2A = _tile([KA, BL, 30, 30], bf16, name="M2A")
        S11A = _tile([KA, BL, 30, 30], bf16, name="S11A")

        # ---- staging / weights ----
        xif = _tile([3, BL, H, W], f32, name="xif")
        tdf = _tile([HD, BL, H, W], f32, name="tdf")
        wraw = _tile([KA, WPK_COLS], f32, name="wraw")
        biasT = _tile([16, 18], f32, name="biasT")
        w100f = _tile([100, 11], f32, name="w100f")
        w1f = _tile([128, 8, 7, 100], f32, name="w1f")
        w1b = _tile([128, 8, 7, 100], bf16, name="w1b")
        w2tb = _tile([100, 10], bf16, name="w2tb")

        wts = {}
        for nm, (off, M) in WREG.items():
            wts[nm] = _tile([KA, M], bf16, name=f"wt_{nm}")

        # ---- activations / temps (fp32, full batch) ----
        Ua = _tile([8, 1568], bf16, name="Ua")
        Ca = _tile([8, 1568], bf16, name="Ca")
        S16b = _tile([16, 1568], bf16, name="S16b")
        Ub8 = _tile([8, 1568], bf16, name="Ub8")
        Sb = _tile([8, 1568], bf16, name="Sb")
        t1 = _tile([8, 1568], bf16, name="t1")
        t2 = _tile([8, 1568], bf16, name="t2")
        Uc = _tile([8, 1568], bf16, name="Uc")
        Cc = _tile([8, 1568], bf16, name="Cc")
        Ud = _tile([8, 1568], bf16, name="Ud")
        Cd = _tile([8, 1568], bf16, name="Cd")
        S2a = _tile([8, 1568], bf16, name="S2a")

        TT = _tile([128, 7, 8, BL], bf16, name="TT")
        ident = _tile([8, 8], bf16, name="ident")
        relu1 = _tile([100, BL], bf16, name="relu1")
        outs = _tile([BL, 10], f32, name="outs")

        # ---- input DMAs (sync ring, dependency order; w1 comes later) ----
        nc.sync.dma_start(out=xif[:], in_=xin_e[:])
        nc.sync.dma_start(out=wraw[:], in_=wpk_e[:])
        nc.sync.dma_start(out=biasT[:], in_=bias_e[:])
        nc.sync.dma_start(out=tdf[:], in_=td_e[:])
        nc.sync.dma_start(out=w100f[:], in_=w100_e[:])

        # ---- preload ACT LUT tables (sigmoid + tanh) before they gate ----
        dummy = _tile([1, 4], f32, name="dummy")
        nc.gpsimd.memset(dummy[:], 0.0)
        nc.scalar.activation(dummy[:], dummy[:], AF.Sigmoid)
        nc.scalar.activation(dummy[:], dummy[:], AF.Tanh)

        # ---- early work: arena memsets + weight casts ----
        nc.vector.memset(XIA[:], 0.0)
        nc.vector.tensor_copy(XIA[0:3, :, 1:29, 1:29], xif[:])

        def wv(nm):
            off, M = WREG[nm]
            return wraw[0:KA, off:off + M]

        def wcast(nm, scale):
            nc.vector.tensor_scalar(wts[nm][:], wv(nm), float(scale), None,
                                    OP.mult)

        wcast("xia", 1.0)
        nc.gpsimd.memset(SGTA[:], 0.0)
        nc.gpsimd.memset(S11A[:], 0.0)
        nc.gpsimd.memset(TT[:], 0.0)
        make_identity(nc, ident[:])

        def pad_memsets(arr):
            nc.gpsimd.memset(arr[0:8, :, 0, :], 0.0)
            nc.gpsimd.memset(arr[0:8, :, 29, :], 0.0)
            nc.gpsimd.memset(arr[0:8, :, 1:29, 0], 0.0)
            nc.gpsimd.memset(arr[0:8, :, 1:29, 29], 0.0)

        for arr in (XA, HA, RA, S01A):
            pad_memsets(arr)

        wcast("a0u", 0.5)
        wcast("a0c", 1.0)
        wcast("g01x", 0.5)
        wcast("g01h", 0.5)
        wcast("c01x", 1.0)
        wcast("c01r", 1.0)
        wcast("a1u", 0.4)
        wcast("a1c", 0.8)
        wcast("m2u", 0.7)
        wcast("s11c", 0.7)
        nc.vector.tensor_copy(w2tb[:], w100f[0:100, 0:10])

        # ---- helpers ----
        DMA_BLOCKS = [1, 2, 3, 5, 6, 7]  # tap indices filled by DMA
        DVE_BLOCKS = [4, 8]              # taps at rows 32/64: DVE copies
        SHIFT_ENGINES = [nc.sync, nc.gpsimd, nc.scalar, nc.sync, nc.gpsimd,
                         nc.sync]

        def mid_int(arr):
            return arr[0:8, :, 1:29, 1:29]

        def shifts(arr):
            flat = arr.rearrange("p b r w -> p (b r w)")
            n = BL * 900
            for ei, k in enumerate(DMA_BLOCKS):
                dy, dx = TAPS[k]
                s = 30 * (dy - 1) + (dx - 1)
                L = n - abs(s)
                d0 = max(0, -s)
                s0 = max(0, s)
                p = BLOCK_OFF[k]
                SHIFT_ENGINES[ei].dma_start(out=flat[p:p + 8, d0:d0 + L],
                                            in_=flat[0:8, s0:s0 + L])
            for k in DVE_BLOCKS:
                dy, dx = TAPS[k]
                s = 30 * (dy - 1) + (dx - 1)
                L = n - abs(s)
                d0 = max(0, -s)
                s0 = max(0, s)
                p = BLOCK_OFF[k]
                nc.vector.tensor_copy(flat[p:p + 8, d0:d0 + L],
                                      flat[0:8, s0:s0 + L])

        def conv(ps, groups):
            """groups: list of (arena, wt_name, row0, M).  Iterate groups
            outer / chunks inner so the ACT consuming the first group's psum
            rows can start while later groups' matmuls still stream."""
            seen = {}
            for (arena, wnm, row0, M) in groups:
                seen.setdefault(row0, []).append((arena, wnm, M))
            for row0, glist in seen.items():
                for gi, (arena, wnm, M) in enumerate(glist):
                    for ci in range(4):
                        bi, h0 = ci // 2, (ci % 2) * 14
                        nc.tensor.matmul(
                            ps[row0:row0 + M, ci, 0:392],
                            wts[wnm][:, :],
                            arena[:, bi, 1 + h0:15 + h0, 1:29],
                            start=(gi == 0), stop=(gi == len(glist) - 1),
                        )

        def psin(ps, p0, p1):
            return ps[p0:p1, :, 0:392]

        with tc.tile_pool(name="cps", bufs=2, space="PSUM") as cps:
            # ---- input conv ----
            shifts(XIA)
            ps0 = cps.tile([40, 4, 512], f32, tag="cp", name="ps0")
            conv(ps0, [(XIA, "xia", 0, 8)])
            nc.scalar.activation(mid_int(XA), psin(ps0, 0, 8),
                                 AF.Identity, bias=biasT[0:8, 0:1])
            shifts(XA)

            # topdown sigmoid arena (off critical path; needed by GRU1 tail)
            nc.scalar.activation(SGTA[0:8, :, 1:29, 1:29], tdf[:], AF.Sigmoid)
            shifts(SGTA)

            # ---- GRU0 sweep0: s00 = u * cand ----
            ps1 = cps.tile([40, 4, 512], f32, tag="cp", name="ps1")
            conv(ps1, [(XA, "a0u", 0, 8), (XA, "a0c", 32, 8)])
            nc.scalar.activation(Ua[:], psin(ps1, 0, 8), AF.Sigmoid,
                                 bias=biasT[0:8, 1:2])
            nc.scalar.activation(Ca[:], psin(ps1, 32, 40), AF.Tanh,
                                 bias=biasT[0:8, 3:4])
            nc.vector.tensor_tensor(mid_int(HA), Ua[:], Ca[:], OP.mult)
            shifts(HA)

            # ---- GRU0 sweep1 gates (fused [r|u]; u extracted by DMA,
            #      consumed only after the cand conv) ----
            ps2 = cps.tile([40, 4, 512], f32, tag="cp", name="ps2")
            conv(ps2, [(XA, "g01x", 0, 16), (HA, "g01h", 0, 16)])
            nc.scalar.activation(S16b[:], psin(ps2, 0, 16), AF.Sigmoid,
                                 bias=biasT[0:16, 2:3])
            nc.sync.dma_start(out=Ub8[:], in_=S16b[8:16, :])
            nc.vector.tensor_tensor(mid_int(RA), S16b[0:8, :], mid_int(HA),
                                    OP.mult)
            shifts(RA)
            # stream in the first half of fc1 weights while PE is busy
            nc.sync.dma_start(out=w1f[0:64, :, :, :], in_=w1_e[0:64, :, :, :])

            # ---- GRU0 sweep1 cand + update ----
            ps3 = cps.tile([40, 4, 512], f32, tag="cp", name="ps3")
            conv(ps3, [(XA, "c01x", 0, 8), (RA, "c01r", 0, 8)])
            nc.scalar.activation(Sb[:], psin(ps3, 0, 8), AF.Tanh,
                                 bias=biasT[0:8, 3:4])
            nc.vector.tensor_tensor(t1[:], Sb[:], mid_int(HA), OP.subtract)
            nc.vector.tensor_tensor(t2[:], Ub8[:], t1[:], OP.mult)
            nc.vector.tensor_tensor(mid_int(S01A), mid_int(HA), t2[:], OP.add)
            shifts(S01A)

            # ---- GRU1 sweep1 ----
            ps4 = cps.tile([40, 4, 512], f32, tag="cp", name="ps4")
            conv(ps4, [(S01A, "a1u", 0, 8), (S01A, "a1c", 32, 8)])
            nc.scalar.activation(Uc[:], psin(ps4, 0, 8), AF.Sigmoid,
                                 bias=biasT[0:8, 4:5])
            nc.scalar.activation(Cc[:], psin(ps4, 32, 40), AF.Tanh,
                                 bias=biasT[0:8, 5:6])
            nc.vector.tensor_tensor(mid_int(S11A), Uc[:], Cc[:], OP.mult)
            shifts(S11A)
            # m-arena = s11-arena * sigmoid(td)-arena, all taps at once
            nc.vector.tensor_tensor(M2A[0:KA, :, :, :], S11A[0:KA, :, :, :],
                                    SGTA[0:KA, :, :, :], OP.mult)
            nc.sync.dma_start(out=w1f[64:128, :, :, :], in_=w1_e[64:128, :, :, :])
            nc.vector.tensor_copy(w1b[:], w1f[:])

            # ---- GRU2 sweep1 (u from m; cand from s11) ----
            ps5 = cps.tile([40, 4, 512], f32, tag="cp", name="ps5")
            conv(ps5, [(M2A, "m2u", 0, 8), (S11A, "s11c", 32, 8)])
            nc.scalar.activation(Ud[:], psin(ps5, 0, 8), AF.Sigmoid,
                                 bias=biasT[0:8, 6:7])
            nc.scalar.activation(Cd[:], psin(ps5, 32, 40), AF.Tanh,
                                 bias=biasT[0:8, 7:8])
            nc.vector.tensor_tensor(S2a[:], Ud[:], Cd[:], OP.mult)

        # ---- FC head (relu folded into the transpose copy-out) ----
        with tc.tile_pool(name="tps", bufs=6, space="PSUM") as tps, \
             tc.tile_pool(name="hps", bufs=1, space="PSUM") as hps:
            p1 = hps.tile([100, BL], f32, tag="p1", name="p1")
            idx = 0
            for r in range(7):
                n = 128 if r < 6 else 784 - 6 * 128
                for b in range(BL):
                    tp = tps.tile([128, 8], bf16, tag="tp", name=f"tp{b}{r}")
                    nc.tensor.transpose(
                        tp[0:n, 0:8],
                        S2a[:, b * 784 + 128 * r: b * 784 + 128 * r + n],
                        ident[:])
                    if b == 0:
                        nc.scalar.activation(TT[0:n, r, :, b], tp[0:n, 0:8],
                                             AF.Relu)
                    else:
                        nc.vector.tensor_scalar_max(TT[0:n, r, :, b],
                                                    tp[0:n, 0:8], 0.0)
                for c8 in range(8):
                    nc.tensor.matmul(
                        p1[:, :],
                        w1b[:, c8, r, :],
                        TT[:, r, c8, :],
                        start=(idx == 0), stop=(idx == 55),
                    )
                    idx += 1
            nc.scalar.activation(relu1[:], p1[:], AF.Relu,
                                 bias=w100f[0:100, 10:11])
            p2 = hps.tile([BL, 10], f32, tag="p2", name="p2")
            nc.tensor.matmul(p2[:, :], relu1[:], w2tb[:], start=True, stop=True)
            nc.vector.tensor_tensor(outs[:], p2[:, :], biasT[0:BL, 8:18],
                                    OP.add)

        nc.sync.dma_start(out=out_e[:], in_=outs[:])

    nc.finalize()
    return nc


def prep_shared(inputs):
    f = lambda k: np.ascontiguousarray(np.asarray(inputs[k], np.float32))
    input_conv_w = f("input_conv_w")
    gates_w = f("gates_w")
    can_w = f("can_w")
    gates_b = f("gates_b")
    can_b = f("can_b")
    input_conv_b = f("input_conv_b")
    fc1_w = f("fc1_w")
    fc1_b = f("fc1_b")
    fc2_w = f("fc2_w")
    fc2_b = f("fc2_b")

    def re9(w):
        # (O, C<=8, 3, 3) -> (KA, O): tap k's rows at BLOCK_OFF[k]
        O, C = w.shape[0], w.shape[1]
        a = w.transpose(2, 3, 1, 0)  # (ky, kx, c, o)
        out = np.zeros((KA, O), np.float32)
        for k, (dy, dx) in enumerate(TAPS):
            out[BLOCK_OFF[k]:BLOCK_OFF[k] + C] = a[dy, dx]
        return out

    wpk = np.zeros((KA, WPK_COLS), np.float32)

    def put(nm, arr):
        off, M = WREG[nm]
        assert arr.shape == (KA, M), (nm, arr.shape)
        wpk[:, off:off + M] = arr

    put("xia", re9(input_conv_w))
    put("a0u", re9(gates_w[0][8:16, :8]))
    put("a0c", re9(can_w[0][:, :8]))
    put("g01x", re9(gates_w[0][:, 0:8]))
    put("g01h", re9(gates_w[0][:, 8:16]))
    put("c01x", re9(can_w[0][:, 0:8]))
    put("c01r", re9(can_w[0][:, 8:16]))
    put("a1u", re9(gates_w[1][8:16, :8]))
    put("a1c", re9(can_w[1][:, :8]))
    put("m2u", re9(gates_w[2][8:16, 0:8]))
    put("s11c", re9(can_w[2][:, 0:8]))

    biasp = np.zeros((16, 18), np.float32)
    biasp[0:8, 0] = input_conv_b
    biasp[0:8, 1] = gates_b[0][8:16]
    biasp[0:8, 2] = gates_b[0][0:8]
    biasp[8:16, 2] = gates_b[0][8:16]
    biasp[0:8, 3] = can_b[0]
    biasp[0:8, 4] = gates_b[1][8:16]
    biasp[0:8, 5] = can_b[1]
    biasp[0:8, 6] = gates_b[2][8:16]
    biasp[0:8, 7] = can_b[2]
    biasp[0:BL, 8:18] = fc2_b[None, :]

    w100 = np.zeros((100, 11), np.float32)
    w100[:, 0:10] = fc2_w.T
    w100[:, 10] = fc1_b

    w1r = fc1_w.reshape(100, 8, 784)
    w1h = np.zeros((128, 8, 7, 100), np.float32)
    for r in range(7):
        n = min(128, 784 - 128 * r)
        w1h[:n, :, r, :] = w1r[:, :, 128 * r:128 * r + n].transpose(2, 1, 0)

    return dict(wpk=wpk, biasp=biasp, w100=w100,
                w1h=np.ascontiguousarray(w1h))


def _fast_path_ok(inputs):
    z = lambda k: not np.any(np.asarray(inputs[k]))
    return (z("td_b0") and z("td_b1")
            and not np.any(np.asarray(inputs["can_b"])[1])
            and not np.any(np.asarray(inputs["can_b"])[2]))


def kernel(**inputs):
    global LAST_EXEC_NS, LAST_TRACE_DIR, LAST_RESULTS
    from concourse.bass_utils import run_bass_kernel_spmd

    if not _fast_path_ok(inputs):
        raise NotImplementedError(
            "general-bias path not implemented (spec guarantees zero biases)")

    if "nc" not in _CACHE:
        _CACHE["nc"] = build_fast_nc()
    nc = _CACHE["nc"]

    shared = prep_shared(inputs)
    it = np.asarray(inputs["input_tensor"], np.float32)
    td = np.asarray(inputs["topdown_input"], np.float32)

    in_maps = []
    for c in range(NCORES):
        b0 = c * BL
        xin = np.ascontiguousarray(
            it[b0:b0 + BL, :, 0].transpose(1, 0, 2, 3))        # (3, BL, H, W)
        td8 = np.ascontiguousarray(
            td[b0:b0 + BL, :HD].transpose(1, 0, 2, 3))         # (HD, BL, H, W)
        in_maps.append(dict(xin=xin, td8=td8, **shared))

    trace = bool(int(os.environ.get("KBENCH_TRACE", "0")))
    tmpdir = None
    if trace:
        import tempfile
        tmpdir = tempfile.mkdtemp(prefix="kbench_trace_")
    res = run_bass_kernel_spmd(nc, in_maps, core_ids=list(range(NCORES)),
                               trace=trace, tmpdir=tmpdir)
    LAST_EXEC_NS = res.exec_time_ns
    LAST_TRACE_DIR = tmpdir
    LAST_RESULTS = res
    out = np.concatenate([np.asarray(r["out"], np.float32)
                          for r in res.results], 0)
    return out
